# revision 29
# baseline (speedup 1.0000x reference)
"""Multi-head self-attention Trainium2 kernel (8 NeuronCores).

Problem: B=4, S=2048, E=1024, 16 heads x 64 dim, fp32 reference.
    Q = x@Wq+bq; K = x@Wk+bk; V = x@Wv+bv   (weights [in, out])
    attn = softmax(Q K^T / sqrt(64)) V      per (batch, head)
    out  = attn@Wo + bo

Sharding: 8 cores = (batch b, head-half hg). Core c handles batch c//2 and
heads 8*(c%2) .. 8*(c%2)+8 (columns 512*hg .. 512*hg+512 of the QKV
projections, rows 512*hg.. of Wo). Each core produces a partial
[2048, 1024] output contribution; host sums the two half-head partials
per batch and adds bo_eff = bo + bv@Wo (softmax rows sum to 1, so
attn@bv == bv and the V bias can be folded into the output bias on the
host - the kernel never touches bv).

All matmuls run in bf16 (inputs cast on host), accumulating in fp32
PSUM; tolerance is 2e-2 and bf16 keeps us ~5e-3. Per-core dataflow:
  Phase 0: x^T (bf16) loaded once, resident in SBUF.
  Phase 1: K^T [512,2048] and V1 (per-head V columns + ones column for
           softmax sums) from x^T tiles.
  Phase 2 per 512-query block: Q^T block, then per head:
           S^T[k,q] = K @ Q^T (two 128-token chunks into one 2-bank
           PSUM tile), one exp on ACT per pair (scale=1/8, no max
           subtraction - scores are O(5) so fp32 exp is exact enough),
           U^T[65,512] = V1^T @ E^T accumulated over chunks. Row 64 of
           U^T = softmax denominators. A^T = U^T[:64] * (1/denominator)
           in bf16. Then the output projection A @ Wo per query block.
"""

import os
import sys

sys.path.insert(0, "/opt/trn_rl_repo")

import numpy as np

B, S, E = 4, 2048, 1024
H = 8           # heads per core
D = 64          # head dim
HC = 512        # projection columns per core
EC = E // 128   # embed chunks (8)
CC = HC // 128  # col chunks (4)
NB = S // 512   # 512-token blocks (4)
TC = S // 128   # 128-token chunks (16)

_CACHE = {}
LAST_RESULTS = None


def _build():
    import concourse.bacc as bacc
    import concourse.tile as tile
    from concourse import mybir

    FP32 = mybir.dt.float32
    BF16 = mybir.dt.bfloat16
    Exp = mybir.ActivationFunctionType.Exp

    nc = bacc.Bacc("TRN2", target_bir_lowering=False, debug=False,
                   enable_asserts=True, num_devices=8)

    xt_d = nc.dram_tensor("xt", [E, S], BF16, kind="ExternalInput").ap()
    wq_d = nc.dram_tensor("wq", [E, HC], BF16, kind="ExternalInput").ap()
    wk_d = nc.dram_tensor("wk", [E, HC], BF16, kind="ExternalInput").ap()
    wv_d = nc.dram_tensor("wv", [E, HC], BF16, kind="ExternalInput").ap()
    wo_d = nc.dram_tensor("wo", [HC, E], BF16, kind="ExternalInput").ap()
    bq_d = nc.dram_tensor("bq", [HC], FP32, kind="ExternalInput").ap()
    bk_d = nc.dram_tensor("bk", [HC], FP32, kind="ExternalInput").ap()
    out_d = nc.dram_tensor("out", [S, E], FP32, kind="ExternalOutput").ap()

    with tile.TileContext(nc) as tc:
        from contextlib import ExitStack
        with ExitStack() as ctx:
            pers = ctx.enter_context(tc.tile_pool(name="pers", bufs=1))
            at_pool = ctx.enter_context(tc.tile_pool(name="at", bufs=2))
            r_pool = ctx.enter_context(tc.tile_pool(name="rp", bufs=6))
            a2_pool = ctx.enter_context(tc.tile_pool(name="a2p", bufs=6))
            o_pool = ctx.enter_context(tc.tile_pool(name="op", bufs=3))
            ps_s = ctx.enter_context(
                tc.tile_pool(name="pss", bufs=3, space="PSUM"))
            ps_m = ctx.enter_context(
                tc.tile_pool(name="psm", bufs=2, space="PSUM"))

            # ---- persistent inputs ----
            wq_sb = pers.tile([128, EC, HC], BF16)
            wk_sb = pers.tile([128, EC, HC], BF16)
            wv_sb = pers.tile([128, EC, HC], BF16)
            wo_sb = pers.tile([128, CC, E], BF16)
            bq_sb = pers.tile([128, CC], FP32)
            bk_sb = pers.tile([128, CC], FP32)
            kt_sb = pers.tile([128, CC, S], BF16)           # K^T [col, tok]
            qt_sb = pers.tile([128, NB, CC, 512], BF16)     # Q^T all blocks
            v1_sb = pers.tile([128, TC, H, D + 1], BF16)    # V + ones col
            ones_sb = pers.tile([128, 1], BF16)

            nc.gpsimd.dma_start(bk_sb[:], bk_d.rearrange("(c p) -> p c", p=128))
            nc.gpsimd.dma_start(bq_sb[:], bq_d.rearrange("(c p) -> p c", p=128))
            nc.vector.memset(ones_sb[:], 1.0)
            nc.vector.tensor_copy(
                v1_sb[:, :, :, D:D + 1],
                ones_sb[:].to_broadcast((128, TC, H, 1)))

            # ---- phase 1: all projections up front (x^T lives only here,
            # its SBUF is released to the exp-tile ring afterwards) ----
            with tc.tile_pool(name="xtp", bufs=1) as xt_pool:
                xt_sb = xt_pool.tile([128, EC, S], BF16)
                for e in range(EC):
                    if e % 2 == 0:
                        nc.sync.dma_start(
                            xt_sb[:, e, 0:512],
                            xt_d[e * 128:(e + 1) * 128, 0:512])
                    else:
                        nc.gpsimd.dma_start(
                            xt_sb[:, e, 0:512],
                            xt_d[e * 128:(e + 1) * 128, 0:512])
                    nc.scalar.dma_start(
                        wk_sb[:, e, :], wk_d[e * 128:(e + 1) * 128, :])
                for tb in range(1, NB):
                    for e in range(EC):
                        nc.sync.dma_start(
                            xt_sb[:, e, tb * 512:(tb + 1) * 512],
                            xt_d[e * 128:(e + 1) * 128, tb * 512:(tb + 1) * 512])
                for e in range(EC):
                    nc.scalar.dma_start(
                        wq_sb[:, e, :], wq_d[e * 128:(e + 1) * 128, :])
                    nc.gpsimd.dma_start(
                        wv_sb[:, e, :], wv_d[e * 128:(e + 1) * 128, :])
                for dchunk in range(CC):
                    nc.gpsimd.dma_start(wo_sb[:, dchunk, :],
                                        wo_d[dchunk * 128:(dchunk + 1) * 128, :])

                for tb in range(NB):
                    for cc in range(CC):
                        ps = ps_m.tile([128, 512], FP32, tag="m", name="kps")
                        for e in range(EC):
                            nc.tensor.matmul(
                                ps[:],
                                wk_sb[:, e, cc * 128:(cc + 1) * 128],
                                xt_sb[:, e, tb * 512:(tb + 1) * 512],
                                start=(e == 0), stop=(e == EC - 1))
                        nc.vector.tensor_scalar_add(
                            kt_sb[:, cc, tb * 512:(tb + 1) * 512], ps[:],
                            bk_sb[:, cc:cc + 1])
                for qb in range(NB):
                    for cc in range(CC):
                        ps = ps_m.tile([128, 512], FP32, tag="m", name="qps")
                        for e in range(EC):
                            nc.tensor.matmul(
                                ps[:],
                                wq_sb[:, e, cc * 128:(cc + 1) * 128],
                                xt_sb[:, e, qb * 512:(qb + 1) * 512],
                                start=(e == 0), stop=(e == EC - 1))
                        nc.vector.tensor_scalar_add(
                            qt_sb[:, qb, cc, :], ps[:], bq_sb[:, cc:cc + 1])
                for tb in range(NB):
                    for t in range(4):
                        ps = ps_m.tile([128, 512], FP32, tag="m", name="vps")
                        for e in range(EC):
                            nc.tensor.matmul(
                                ps[:],
                                xt_sb[:, e,
                                      tb * 512 + t * 128:tb * 512 + (t + 1) * 128],
                                wv_sb[:, e, :],
                                start=(e == 0), stop=(e == EC - 1))
                        nc.vector.tensor_copy(
                            v1_sb[:, tb * 4 + t, :, 0:D],
                            ps[:].rearrange("p (h d) -> p h d", h=H))

            # ---- phase 2: ACT-paced attention stream with a 2-pair exp
            # buffer so PV never catches an unfinished exp ----
            with tc.tile_pool(name="ep", bufs=48) as e_pool:

                def s_exp_head(qb, h):
                    p0 = 64 * (h % 2)
                    cc = h // 2
                    es = []
                    for kcp in range(TC // 2):
                        s_ps = ps_s.tile([128, 1024], FP32)
                        for half in range(2):
                            kc = 2 * kcp + half
                            nc.tensor.matmul(
                                s_ps[:, half * 512:(half + 1) * 512],
                                kt_sb[p0:p0 + D, cc,
                                      kc * 128:(kc + 1) * 128],
                                qt_sb[p0:p0 + D, qb, cc, :],
                                start=True, stop=True)
                        e_t = e_pool.tile([128, 1024], BF16)
                        nc.scalar.activation(e_t[:], s_ps[:], Exp,
                                             bias=0.0, scale=0.125)
                        es.append(e_t)
                    return es

                def pv_norm_pair(qb, hp, es0, es1, at_sb):
                    for qc in range(4):
                        a2 = a2_pool.tile([128, 128], BF16)
                        u2a = ps_m.tile([128, D + 1], FP32, tag="m",
                                        name="u2a")
                        u2b = ps_m.tile([128, D + 1], FP32, tag="m",
                                        name="u2b")
                        for kcp in range(TC // 2):
                            for half in range(2):
                                kc = 2 * kcp + half
                                sl = slice(half * 512 + qc * 128,
                                           half * 512 + (qc + 1) * 128)
                                nc.tensor.matmul(
                                    u2a[:], es0[kcp][:, sl],
                                    v1_sb[:, kc, 2 * hp, :],
                                    start=(kc == 0), stop=(kc == TC - 1))
                                nc.tensor.matmul(
                                    u2b[:], es1[kcp][:, sl],
                                    v1_sb[:, kc, 2 * hp + 1, :],
                                    start=(kc == 0), stop=(kc == TC - 1))
                        for sub, u2 in ((0, u2a), (1, u2b)):
                            r_t = r_pool.tile([128, 1], FP32)
                            nc.vector.reciprocal(r_t[:], u2[:, D:D + 1])
                            nc.vector.tensor_scalar_mul(
                                a2[:, sub * D:(sub + 1) * D], u2[:, 0:D],
                                r_t[:])
                        nc.sync.dma_start_transpose(
                            at_sb[:, hp, qc * 128:(qc + 1) * 128], a2[:])

                def out_proj(qb, at_sb):
                    for qc in range(4):
                        for eb in range(2):
                            ps = ps_m.tile([128, 512], FP32, tag="m",
                                           name="ops")
                            for dchunk in range(CC):
                                nc.tensor.matmul(
                                    ps[:],
                                    at_sb[:, dchunk,
                                          qc * 128:(qc + 1) * 128],
                                    wo_sb[:, dchunk,
                                          eb * 512:(eb + 1) * 512],
                                    start=(dchunk == 0),
                                    stop=(dchunk == CC - 1))
                            o_t = o_pool.tile([128, 512], FP32)
                            nc.vector.tensor_copy(o_t[:], ps[:])
                            nc.gpsimd.dma_start(
                                out_d[qb * 512 + qc * 128:
                                      qb * 512 + (qc + 1) * 128,
                                      eb * 512:(eb + 1) * 512],
                                o_t[:])

                ats = {}
                state = {"out": None}
                pend = []  # pairs with S/exp emitted, awaiting PV

                def pv_slot(ctx2, j):
                    """Emit 8 PV matmuls (slot j of 16) for a pending pair;
                    every 4th slot closes a qc group with norm + XBAR."""
                    (fqb, fhp, es0, es1) = ctx2["pair"]
                    qc = j // 4
                    k4 = j % 4
                    if k4 == 0:
                        ctx2["a2"] = a2_pool.tile([128, 128], BF16, name="a2")
                        ctx2["u2a"] = ps_m.tile([128, D + 1], FP32, tag="m",
                                                name="u2a")
                        ctx2["u2b"] = ps_m.tile([128, D + 1], FP32, tag="m",
                                                name="u2b")
                    for kcp in (2 * k4, 2 * k4 + 1):
                        for half in range(2):
                            kc = 2 * kcp + half
                            sl = slice(half * 512 + qc * 128,
                                       half * 512 + (qc + 1) * 128)
                            nc.tensor.matmul(
                                ctx2["u2a"][:], es0[kcp][:, sl],
                                v1_sb[:, kc, 2 * fhp, :],
                                start=(kc == 0), stop=(kc == TC - 1))
                            nc.tensor.matmul(
                                ctx2["u2b"][:], es1[kcp][:, sl],
                                v1_sb[:, kc, 2 * fhp + 1, :],
                                start=(kc == 0), stop=(kc == TC - 1))
                    if k4 == 3:
                        for sub, u2 in ((0, ctx2["u2a"]), (1, ctx2["u2b"])):
                            r_t = r_pool.tile([128, 1], FP32)
                            nc.vector.reciprocal(r_t[:], u2[:, D:D + 1])
                            nc.vector.tensor_scalar_mul(
                                ctx2["a2"][:, sub * D:(sub + 1) * D],
                                u2[:, 0:D], r_t[:])
                        nc.sync.dma_start_transpose(
                            ats[fqb][:, fhp, qc * 128:(qc + 1) * 128],
                            ctx2["a2"][:])

                pairs = [(qb, hp) for qb in range(NB) for hp in range(4)]
                for qb, hp in pairs:
                    if qb not in ats:
                        ats[qb] = at_pool.tile([128, CC, 512], BF16,
                                               name="at_sb")
                    if state["out"] is not None:
                        out_proj(*state["out"])
                        state["out"] = None
                    # S/exp for this pair, interleaved slot-by-slot with PV
                    # of the pair two behind (all its exps are complete, so
                    # these matmuls never wait and keep the tensor engine
                    # busy while the s-tile ring drains at ACT's exp pace).
                    pv_ctx = {"pair": pend[0]} if len(pend) >= 2 else None
                    es0, es1 = [], []
                    for j in range(16):
                        h = 2 * hp + (j // 8)
                        kcp = j % 8
                        p0 = 64 * (h % 2)
                        cc = h // 2
                        s_ps = ps_s.tile([128, 1024], FP32)
                        for half in range(2):
                            kc = 2 * kcp + half
                            nc.tensor.matmul(
                                s_ps[:, half * 512:(half + 1) * 512],
                                kt_sb[p0:p0 + D, cc,
                                      kc * 128:(kc + 1) * 128],
                                qt_sb[p0:p0 + D, qb, cc, :],
                                start=True, stop=True)
                        e_t = e_pool.tile([128, 1024], BF16)
                        nc.scalar.activation(e_t[:], s_ps[:], Exp,
                                             bias=0.0, scale=0.125)
                        (es0 if j < 8 else es1).append(e_t)
                        if pv_ctx is not None:
                            pv_slot(pv_ctx, j)
                    if pv_ctx is not None:
                        fqb, fhp, _, _ = pend.pop(0)
                        if fhp == 3:
                            state["out"] = (fqb, ats[fqb])
                    pend.append((qb, hp, es0, es1))

                while pend:
                    pv_ctx = {"pair": pend[0]}
                    for j in range(16):
                        pv_slot(pv_ctx, j)
                    fqb, fhp, _, _ = pend.pop(0)
                    if state["out"] is not None:
                        out_proj(*state["out"])
                        state["out"] = None
                    if fhp == 3:
                        state["out"] = (fqb, ats[fqb])
                if state["out"] is not None:
                    out_proj(*state["out"])

    nc.compile()
    return nc


def _register_ntff_hook():
    """The image's antenv lacks axon_hooks, so trace=True would die on the
    import inside run_bass_kernel_spmd. Shim the module and register the
    ctypes NTFF hook from trn_boot when tracing is requested."""
    import types

    if "antenv.axon_hooks" in sys.modules:
        return
    mod = types.ModuleType("antenv.axon_hooks")
    _state = {"hook": None}
    mod.set_axon_ntff_profile_hook = lambda h: _state.__setitem__("hook", h)
    mod.get_axon_ntff_profile_hook = lambda: _state["hook"]
    sys.modules["antenv.axon_hooks"] = mod
    try:
        import antenv

        antenv.axon_hooks = mod
    except ImportError:
        pass
    try:
        from trn_agent_boot.trn_boot import _ntff_profile_via_ctypes

        mod.set_axon_ntff_profile_hook(
            _ntff_profile_via_ctypes("/opt/axon/libaxon_pjrt.so"))
    except Exception:
        pass


def kernel(x, Wq, bq, Wk, bk, Wv, bv, Wo, bo):
    global LAST_RESULTS
    import ml_dtypes
    from concourse.bass_utils import run_bass_kernel_spmd

    if "nc" not in _CACHE:
        _CACHE["nc"] = _build()
    nc = _CACHE["nc"]

    bf16 = ml_dtypes.bfloat16
    x = np.asarray(x, dtype=np.float32)
    Wq = np.asarray(Wq, dtype=np.float32)
    Wk = np.asarray(Wk, dtype=np.float32)
    Wv = np.asarray(Wv, dtype=np.float32)
    Wo = np.asarray(Wo, dtype=np.float32)
    in_maps = []
    for c in range(8):
        b, hg = c // 2, c % 2
        sl = slice(HC * hg, HC * hg + HC)
        in_maps.append({
            "xt": np.ascontiguousarray(x[b].T).astype(bf16),
            "wq": np.ascontiguousarray(Wq[:, sl]).astype(bf16),
            "wk": np.ascontiguousarray(Wk[:, sl]).astype(bf16),
            "wv": np.ascontiguousarray(Wv[:, sl]).astype(bf16),
            "wo": np.ascontiguousarray(Wo[sl, :]).astype(bf16),
            "bq": np.ascontiguousarray(np.asarray(bq, dtype=np.float32)[sl]),
            "bk": np.ascontiguousarray(np.asarray(bk, dtype=np.float32)[sl]),
        })

    trace = bool(int(os.environ.get("KERNEL_TRACE", "0")))
    if trace:
        _register_ntff_hook()
    res = run_bass_kernel_spmd(nc, in_maps, list(range(8)), trace=trace)
    LAST_RESULTS = res

    # bv folds into the output bias: softmax rows sum to 1 => attn@bv = bv.
    bo_eff = (np.asarray(bo, dtype=np.float32)
              + np.asarray(bv, dtype=np.float32) @ Wo)
    out = np.empty((B, S, E), dtype=np.float32)
    for b in range(B):
        out[b] = (res.results[2 * b]["out"] + res.results[2 * b + 1]["out"]
                  + bo_eff)
    return out
